# revision 18
# baseline (speedup 1.0000x reference)
"""Bahdanau-attention RNN decoder (greedy argmax feedback) on 8 TRN2 NeuronCores.

Self-contained: kernel(**inputs) takes full inputs, shards batch 8-way,
runs a Bass/Tile kernel per core, gathers the full output [B, O, T-1].

Per-core: S=256 src positions, Bl=32 batch, H=512 hidden, O=64 vocab, 63 steps.
Design (all fp32 — the greedy argmax feedback is numerically chaotic; 16-bit
anywhere in the attention path flips argmaxes and diverges trajectories):
  - enc_proj = enc @ Wa_e + ba precomputed on-device into a DRAM scratch,
    streamed back each step (SBUF cannot hold both enc and enc_proj in fp32).
  - encoder_outputs resident in SBUF as [s%128, b, s//128, h] (context lhsT).
  - score = v . tanh(enc_proj + h Wa_h): broadcast-add split between DVE
    (tensor_scalar per (b,kt)) and GPSIMD (tensor_tensor, otherwise idle);
    tanh on ACT; H-contraction on PE with v as 1-wide moving operand.
  - scores split into two 16-batch groups in separate PSUM banks so exp /
    Z-partial / context matmuls for group 0 overlap the second half of the
    chunk pipeline (and the ep DMA stream).
  - softmax unnormalized; Z via per-group column-sum matmuls (tile_position
    col strips) + pmat combine; context normalized during PSUM evacuation.
  - RNN + logits as fp32 matmuls; one-hot(argmax) via PE transpose +
    reduce_max + is_equal.
  - logits accumulate in SBUF ([O, T, Bl]); single DMA at the end.
"""
import contextlib
import numpy as np

import concourse.bacc as bacc
import concourse.tile as tile
from concourse import mybir
from concourse import bass_utils
from concourse.mybir import ActivationFunctionType as AF, AluOpType as ALU

F32 = mybir.dt.float32
S, B, Bl, H, O, T = 256, 256, 32, 512, 64, 63
KT = H // 128  # 4
NCORES = 8
GP_CHUNKS = (3, 6, 10, 13)  # chunks whose broadcast-add runs on GPSIMD


def _build(T=T, steps=None, num_devices=NCORES):
    if steps is None:
        steps = T
    nc = bacc.Bacc("TRN2", target_bir_lowering=False, debug=False,
                   num_devices=num_devices)
    EI = "ExternalInput"
    enc_l1 = nc.dram_tensor("enc_l1", [128, KT, Bl, S], F32, kind=EI)
    enc_l2 = nc.dram_tensor("enc_l2", [128, Bl, 2, H], F32, kind=EI)
    h0_t = nc.dram_tensor("h0_t", [128, KT, Bl], F32, kind=EI)
    x0_t = nc.dram_tensor("x0_t", [O, Bl], F32, kind=EI)
    wae = nc.dram_tensor("wae", [128, KT, H], F32, kind=EI)
    wah = nc.dram_tensor("wah", [128, KT, H], F32, kind=EI)
    wcat = nc.dram_tensor("wcat", [128, 9, H], F32, kind=EI)
    wo_t = nc.dram_tensor("wo_t", [128, KT, O], F32, kind=EI)
    v_t = nc.dram_tensor("v_t", [128, KT], F32, kind=EI)
    ba_t = nc.dram_tensor("ba_t", [128, KT], F32, kind=EI)
    bcat = nc.dram_tensor("bcat", [128, KT], F32, kind=EI)
    bo_t = nc.dram_tensor("bo_t", [O, 1], F32, kind=EI)
    pmat = nc.dram_tensor("pmat", [64, Bl], F32, kind=EI)
    ident = nc.dram_tensor("ident", [64, 64], F32, kind=EI)
    out = nc.dram_tensor("out", [O, T, Bl], F32, kind="ExternalOutput")
    epdram = nc.dram_tensor("epdram", [128, Bl, KT, S], F32, kind="Internal")

    with tile.TileContext(nc) as tc:
        ctx = contextlib.ExitStack()
        with ctx:
            consts = ctx.enter_context(tc.tile_pool(name="consts", bufs=1))
            enc2p = ctx.enter_context(tc.tile_pool(name="enc2", bufs=1))
            state = ctx.enter_context(tc.tile_pool(name="state", bufs=1))

            wah_sb = consts.tile([128, KT, H], F32)
            wcat_sb = consts.tile([128, 9, H], F32)
            wot_sb = consts.tile([128, KT, O], F32)
            v_sb = consts.tile([128, KT], F32)
            ba_sb = consts.tile([128, KT], F32)
            bcat_sb = consts.tile([128, KT], F32)
            bo_sb = consts.tile([O, 1], F32)
            pmat_sb = consts.tile([64, Bl], F32)
            ident_sb = consts.tile([64, 64], F32)
            ones_col = consts.tile([128, 1], F32)
            ones_row = consts.tile([1, 128], F32)
            lg_all = consts.tile([O, 4, Bl], F32)
            u_sb = consts.tile([128, 2 * Bl], F32)
            zinv = consts.tile([128, Bl], F32)
            nc.sync.dma_start(out=wah_sb[:], in_=wah.ap())
            nc.sync.dma_start(out=wcat_sb[:], in_=wcat.ap())
            nc.sync.dma_start(out=wot_sb[:], in_=wo_t.ap())
            nc.sync.dma_start(out=v_sb[:], in_=v_t.ap())
            nc.sync.dma_start(out=ba_sb[:], in_=ba_t.ap())
            nc.sync.dma_start(out=bcat_sb[:], in_=bcat.ap())
            nc.sync.dma_start(out=bo_sb[:], in_=bo_t.ap())
            nc.sync.dma_start(out=pmat_sb[:], in_=pmat.ap())
            nc.sync.dma_start(out=ident_sb[:], in_=ident.ap())
            nc.vector.memset(ones_col[:], 1.0)
            nc.vector.memset(ones_row[:], 1.0)

            enc2_sb = enc2p.tile([128, Bl, 2, H], F32)
            nc.sync.dma_start(out=enc2_sb[:], in_=enc_l2.ap())

            rnn_in = state.tile([128, 9, Bl], F32)
            nc.sync.dma_start(out=rnn_in[:, 4:8, :], in_=h0_t.ap())
            nc.sync.dma_start(out=rnn_in[0:O, 8, :], in_=x0_t.ap())
            nc.vector.memset(rnn_in[O:128, 8, :], 0.0)
            hproj = state.tile([128, KT, Bl], F32)
            hx_sb = state.tile([128, KT, Bl], F32)
            zsb = state.tile([64, 1], F32)
            zrow = state.tile([1, Bl], F32)
            lgb = state.tile([Bl, O], F32)
            mx = state.tile([Bl, 1], F32)
            oh = state.tile([Bl, O], F32)

            # enc_proj = enc @ Wa_e + ba  ->  epdram
            with tc.tile_pool(name="pre", bufs=1) as prep, \
                 tc.tile_pool(name="pre_s", bufs=3) as pres, \
                 tc.tile_pool(name="pre_ps", bufs=4, space="PSUM") as preps:
                wae_sb = prep.tile([128, KT, H], F32)
                nc.sync.dma_start(out=wae_sb[:], in_=wae.ap())
                for j in range(Bl // 2):
                    e1 = pres.tile([128, KT, 2, S], F32, tag="e1")
                    nc.sync.dma_start(out=e1[:], in_=enc_l1.ap()[:, :, 2 * j:2 * j + 2, :])
                    for mo in range(4):
                        pps = preps.tile([128, 512], F32, tag="pps")
                        for kt in range(KT):
                            nc.tensor.matmul(
                                pps[:],
                                wae_sb[:, kt, mo * 128:(mo + 1) * 128],
                                e1[:, kt, :, :].rearrange("p b s -> p (b s)"),
                                start=(kt == 0), stop=(kt == KT - 1))
                        eo = pres.tile([128, 512], F32, tag="eo")
                        nc.vector.tensor_scalar_add(eo[:], pps[:], ba_sb[:, mo:mo + 1])
                        nc.sync.dma_start(
                            out=epdram.ap()[:, 2 * j:2 * j + 2, mo, :],
                            in_=eo[:].rearrange("p (b s) -> p b s", b=2))

            ps_sc = ctx.enter_context(tc.tile_pool(name="ps_sc", bufs=1, space="PSUM"))
            ps_z = ctx.enter_context(tc.tile_pool(name="ps_z", bufs=1, space="PSUM"))
            ps_ctx = ctx.enter_context(tc.tile_pool(name="ps_c", bufs=1, space="PSUM"))
            ps_mm = ctx.enter_context(tc.tile_pool(name="ps_m", bufs=2, space="PSUM"))
            ps_sm = ctx.enter_context(tc.tile_pool(name="ps_sm", bufs=2, space="PSUM"))
            stream = ctx.enter_context(tc.tile_pool(name="stream", bufs=6))

            for t in range(steps):
                # --- hproj = Wa_h^T h ; hx = W_hh h + W_ih[:, :O] x ---
                hp_ps = ps_mm.tile([128, KT, Bl], F32, tag="mm")
                for mo in range(4):
                    for kt in range(KT):
                        nc.tensor.matmul(
                            hp_ps[:, mo, :],
                            wah_sb[:, kt, mo * 128:(mo + 1) * 128],
                            rnn_in[:, 4 + kt, :],
                            start=(kt == 0), stop=(kt == KT - 1))
                nc.vector.tensor_copy(hproj[:], hp_ps[:])

                hx_ps = ps_mm.tile([128, KT, Bl], F32, tag="mm")
                for mo in range(4):
                    for kt in (4, 5, 6, 7, 8):
                        nc.tensor.matmul(
                            hx_ps[:, mo, :],
                            wcat_sb[:, kt, mo * 128:(mo + 1) * 128],
                            rnn_in[:, kt, :],
                            start=(kt == 4), stop=(kt == 8))
                nc.vector.tensor_copy(hx_sb[:], hx_ps[:])

                # --- chunk pipeline: energy -> tanh -> score -----------------
                # processing position pos covers column-batches c = 2*pos+bi;
                # batch b = (4 + c) % 32 so the two res_ep chunks (b 0..3) are
                # processed LAST -- stream-ring buffers then free early enough
                # for 3 prefetch DMAs of step t+1 to overlap this step's tail.
                sc_ps = [ps_sc.tile([128, 512], F32, tag="scA", name="scA"),
                         ps_sc.tile([128, 512], F32, tag="scB", name="scB")]
                ctx_ps = ps_ctx.tile([128, KT, Bl], F32, tag="ctx")
                z_ps = ps_z.tile([64, 1], F32, tag="z")
                for pos in range(16):
                    g = pos // 8
                    w = stream.tile([128, 2, KT, S], F32, tag="ep")
                    nc.sync.dma_start(out=w[:],
                                      in_=epdram.ap()[:, 2 * pos:2 * pos + 2, :, :])
                    src_ap = w[:]
                    for bi in range(2):
                        c = 2 * pos + bi
                        b = c
                        for kt in range(KT):
                            nc.vector.tensor_scalar_add(
                                w[:, bi, kt, :], src_ap[:, bi, kt, :],
                                hproj[:, kt, b:b + 1])
                    nc.scalar.activation(w[:], w[:], AF.Tanh)
                    for bi in range(2):
                        c = 2 * pos + bi
                        col = 4 * (pos % 8) + 2 * bi
                        for s1 in range(2):
                            for kt in range(KT):
                                nc.tensor.matmul(
                                    sc_ps[g][:, col + s1:col + s1 + 1],
                                    w[:, bi, kt, s1 * 128:(s1 + 1) * 128],
                                    v_sb[:, kt:kt + 1],
                                    start=(kt == 0), stop=(kt == KT - 1))
                    if pos == 7 or pos == 15:
                        # group complete: exp, Z, zinv, context, evac
                        nc.scalar.activation(u_sb[:, 32 * g:32 * g + 32],
                                             sc_ps[g][:, 0:32], AF.Exp)
                        nc.tensor.matmul(z_ps[32 * g:32 * g + 32, :],
                                         u_sb[:, 32 * g:32 * g + 32], ones_col[:],
                                         start=True, stop=True,
                                         tile_position=(0, 32 * g))
                        nc.vector.tensor_copy(zsb[32 * g:32 * g + 32, :],
                                              z_ps[32 * g:32 * g + 32, :])
                        zrow_ps = ps_sm.tile([1, 16], F32, tag="sm")
                        nc.tensor.matmul(zrow_ps[:],
                                         zsb[32 * g:32 * g + 32, :],
                                         pmat_sb[32 * g:32 * g + 32,
                                                 16 * g:16 * g + 16],
                                         start=True, stop=True)
                        nc.vector.tensor_copy(zrow[:, 16 * g:16 * g + 16],
                                              zrow_ps[:])
                        zrep_ps = ps_sm.tile([128, 16], F32, tag="sm")
                        nc.tensor.matmul(zrep_ps[:], ones_row[:],
                                         zrow[:, 16 * g:16 * g + 16],
                                         start=True, stop=True)
                        nc.vector.reciprocal(zinv[:, 16 * g:16 * g + 16],
                                             zrep_ps[:])
                        for cc in range(16 * g, 16 * g + 16):
                            b = cc
                            for hc in range(4):
                                for s1 in range(2):
                                    nc.tensor.matmul(
                                        ctx_ps[:, hc, cc:cc + 1],
                                        enc2_sb[:, b, s1, hc * 128:(hc + 1) * 128],
                                        u_sb[:, 2 * cc + s1:2 * cc + s1 + 1],
                                        start=(s1 == 0), stop=(s1 == 1))
                        # evac + normalize: c runs -> b runs ((4+c)%32)
                        for kt in range(KT):
                            nc.vector.tensor_tensor(
                                out=rnn_in[:, kt, 16 * g:16 * g + 16],
                                in0=ctx_ps[:, kt, 16 * g:16 * g + 16],
                                in1=zinv[:, 16 * g:16 * g + 16], op=ALU.mult)

                # --- RNN ---
                h_ps = ps_mm.tile([128, KT, Bl], F32, tag="mm")
                for mo in range(4):
                    for kt in (0, 1, 2, 3):
                        nc.tensor.matmul(
                            h_ps[:, mo, :],
                            wcat_sb[:, kt, mo * 128:(mo + 1) * 128],
                            rnn_in[:, kt, :],
                            start=(kt == 0), stop=(kt == 3))
                nc.vector.tensor_tensor(out=h_ps[:], in0=h_ps[:], in1=hx_sb[:],
                                        op=ALU.add)
                for mo in range(4):
                    nc.scalar.activation(rnn_in[:, 4 + mo, :], h_ps[:, mo, :],
                                         AF.Tanh, bias=bcat_sb[:, mo:mo + 1])

                # --- logits ---
                te = t % T  # bench builds may run steps > T; wrap the output
                lg_ps = ps_sm.tile([O, Bl], F32, tag="sm")
                for kt in range(KT):
                    nc.tensor.matmul(lg_ps[:], wot_sb[:, kt, :], rnn_in[:, 4 + kt, :],
                                     start=(kt == 0), stop=(kt == KT - 1))
                nc.vector.tensor_scalar_add(lg_all[:, te % 4, :], lg_ps[:], bo_sb[:])
                if te % 4 == 3 or te == T - 1 or t == steps - 1:
                    t0 = (te // 4) * 4
                    nc.sync.dma_start(out=out.ap()[:, t0:te + 1, :],
                                      in_=lg_all[:, 0:te + 1 - t0, :])

                # --- greedy one-hot(argmax) feedback ---
                if t < steps - 1:
                    lgb_ps = ps_sm.tile([Bl, O], F32, tag="sm")
                    nc.tensor.transpose(lgb_ps[:], lg_all[:, te % 4, :],
                                        ident_sb[0:O, 0:O])
                    nc.vector.tensor_copy(lgb[:], lgb_ps[:])
                    nc.vector.tensor_reduce(mx[:], lgb[:], axis=mybir.AxisListType.X,
                                            op=ALU.max)
                    nc.vector.tensor_scalar(out=oh[:], in0=lgb[:], scalar1=mx[:],
                                            scalar2=None, op0=ALU.is_equal)
                    oh_ps = ps_sm.tile([O, Bl], F32, tag="sm")
                    nc.tensor.transpose(oh_ps[:], oh[:], ident_sb[0:Bl, 0:Bl])
                    nc.vector.tensor_copy(rnn_in[0:O, 8, :], oh_ps[:])

    nc.compile()
    return nc


def _prep_core_inputs(enc, h0, x0, Wa, ba, v, W_ih, b_ih, W_hh, b_hh, Wo, bo):
    f = np.float32
    Wa_h, Wa_e = Wa[:H], Wa[H:]
    enc_l1 = np.ascontiguousarray(
        enc.transpose(2, 1, 0).reshape(KT, 128, Bl, S).transpose(1, 0, 2, 3), dtype=f)
    enc_l2 = np.ascontiguousarray(
        enc.reshape(2, 128, Bl, H).transpose(1, 2, 0, 3), dtype=f)
    h0_t = np.ascontiguousarray(h0.T.reshape(KT, 128, Bl).transpose(1, 0, 2), dtype=f)
    x0_t = np.ascontiguousarray(x0.T, dtype=f)
    wae_ = np.ascontiguousarray(Wa_e.reshape(KT, 128, H).transpose(1, 0, 2), dtype=f)
    wah_ = np.ascontiguousarray(Wa_h.reshape(KT, 128, H).transpose(1, 0, 2), dtype=f)
    Wcat = np.zeros((9 * 128, H), dtype=f)
    Wcat[0:H] = W_ih[:, O:O + H].T
    Wcat[H:2 * H] = W_hh.T
    Wcat[2 * H:2 * H + O] = W_ih[:, 0:O].T
    wcat_ = np.ascontiguousarray(Wcat.reshape(9, 128, H).transpose(1, 0, 2), dtype=f)
    wo_ = np.ascontiguousarray(Wo.T.reshape(KT, 128, O).transpose(1, 0, 2), dtype=f)
    v_ = np.ascontiguousarray(np.asarray(v, dtype=f).reshape(KT, 128).T)
    ba_ = np.ascontiguousarray(np.asarray(ba, dtype=f).reshape(KT, 128).T)
    bc = (np.asarray(b_ih, dtype=f) + np.asarray(b_hh, dtype=f))
    bcat_ = np.ascontiguousarray(bc.reshape(KT, 128).T)
    bo_ = np.ascontiguousarray(np.asarray(bo, dtype=f).reshape(O, 1))
    # z partials: group g (batches 16g..16g+15) lands on partitions
    # 32g + 2*j + s1 (j = local batch); pmat combines the s1 pairs.
    pm = np.zeros((64, Bl), dtype=f)
    for g in range(2):
        for j in range(16):
            for s1 in range(2):
                pm[32 * g + 2 * j + s1, 16 * g + j] = 1.0
    return {
        "enc_l1": enc_l1, "enc_l2": enc_l2, "h0_t": h0_t, "x0_t": x0_t,
        "wae": wae_, "wah": wah_, "wcat": wcat_, "wo_t": wo_, "v_t": v_,
        "ba_t": ba_, "bcat": bcat_, "bo_t": bo_, "pmat": pm,
        "ident": np.eye(64, dtype=f),
    }


_NC_CACHE = {}


def _get_nc():
    if "nc" not in _NC_CACHE:
        _NC_CACHE["nc"] = _build()
    return _NC_CACHE["nc"]


def kernel(sos_token, h, encoder_outputs, Wa, ba, v, W_ih, b_ih, W_hh, b_hh,
           Wo, bo):
    sos_token = np.asarray(sos_token, dtype=np.float32)
    h = np.asarray(h, dtype=np.float32)
    encoder_outputs = np.asarray(encoder_outputs, dtype=np.float32)
    nc = _get_nc()
    in_maps = []
    for core in range(NCORES):
        sl = slice(core * Bl, (core + 1) * Bl)
        in_maps.append(_prep_core_inputs(
            encoder_outputs[:, sl], h[0][sl], sos_token[0][sl],
            Wa, ba, v, W_ih, b_ih, W_hh, b_hh, Wo, bo))
    res = bass_utils.run_bass_kernel_spmd(nc, in_maps, core_ids=list(range(NCORES)))
    # per-core out [O, T, Bl] -> full [B, O, T]
    return np.concatenate(
        [res.results[c]["out"].transpose(2, 0, 1) for c in range(NCORES)], axis=0)


# revision 25
# speedup vs baseline: 1.0218x; 1.0218x over previous
"""Bahdanau-attention RNN decoder (greedy argmax feedback) on 8 TRN2 NeuronCores.

Self-contained: kernel(**inputs) takes full inputs, shards batch 8-way,
runs a Bass/Tile kernel per core, gathers the full output [B, O, T-1].

Per-core: S=256 src positions, Bl=32 batch, H=512 hidden, O=64 vocab, 63 steps.
Design (all fp32 — the greedy argmax feedback is numerically chaotic; 16-bit
anywhere in the attention path flips argmaxes and diverges trajectories):
  - enc_proj = enc @ Wa_e + ba precomputed on-device into a DRAM scratch,
    streamed back each step (SBUF cannot hold both enc and enc_proj in fp32).
  - encoder_outputs resident in SBUF as [s%128, b, s//128, h] (context lhsT).
  - score = v . tanh(enc_proj + h Wa_h): broadcast-add split between DVE
    (tensor_scalar per (b,kt)) and GPSIMD (tensor_tensor, otherwise idle);
    tanh on ACT; H-contraction on PE with v as 1-wide moving operand.
  - scores split into two 16-batch groups in separate PSUM banks so exp /
    Z-partial / context matmuls for group 0 overlap the second half of the
    chunk pipeline (and the ep DMA stream).
  - softmax unnormalized; Z via per-group column-sum matmuls (tile_position
    col strips) + pmat combine; context normalized during PSUM evacuation.
  - RNN + logits as fp32 matmuls; one-hot(argmax) via PE transpose +
    reduce_max + is_equal.
  - logits accumulate in SBUF ([O, T, Bl]); single DMA at the end.
"""
import contextlib
import numpy as np

import concourse.bacc as bacc
import concourse.tile as tile
from concourse import mybir
from concourse import bass_utils
from concourse.mybir import ActivationFunctionType as AF, AluOpType as ALU

F32 = mybir.dt.float32
S, B, Bl, H, O, T = 256, 256, 32, 512, 64, 63
KT = H // 128  # 4
NCORES = 8
GP_CHUNKS = (3, 6, 10, 13)  # chunks whose broadcast-add runs on GPSIMD


def _build(T=T, steps=None, num_devices=NCORES):
    if steps is None:
        steps = T
    nc = bacc.Bacc("TRN2", target_bir_lowering=False, debug=False,
                   num_devices=num_devices)
    EI = "ExternalInput"
    enc_l1 = nc.dram_tensor("enc_l1", [128, KT, Bl, S], F32, kind=EI)
    enc_l2 = nc.dram_tensor("enc_l2", [128, Bl, 2, H], F32, kind=EI)
    h0_t = nc.dram_tensor("h0_t", [128, KT, Bl], F32, kind=EI)
    x0_t = nc.dram_tensor("x0_t", [O, Bl], F32, kind=EI)
    wae = nc.dram_tensor("wae", [128, KT, H], F32, kind=EI)
    wah = nc.dram_tensor("wah", [128, KT, H], F32, kind=EI)
    wcat = nc.dram_tensor("wcat", [128, 9, H], F32, kind=EI)
    wo_t = nc.dram_tensor("wo_t", [128, KT, O], F32, kind=EI)
    v_t = nc.dram_tensor("v_t", [128, KT], F32, kind=EI)
    ba_t = nc.dram_tensor("ba_t", [128, KT], F32, kind=EI)
    bcat = nc.dram_tensor("bcat", [128, KT], F32, kind=EI)
    bo_t = nc.dram_tensor("bo_t", [O, 1], F32, kind=EI)
    pmat = nc.dram_tensor("pmat", [64, Bl], F32, kind=EI)
    ident = nc.dram_tensor("ident", [64, 64], F32, kind=EI)
    out = nc.dram_tensor("out", [O, T, Bl], F32, kind="ExternalOutput")
    epdram = nc.dram_tensor("epdram", [128, Bl, KT, S], F32, kind="Internal")

    with tile.TileContext(nc) as tc:
        ctx = contextlib.ExitStack()
        with ctx:
            consts = ctx.enter_context(tc.tile_pool(name="consts", bufs=1))
            enc2p = ctx.enter_context(tc.tile_pool(name="enc2", bufs=1))
            state = ctx.enter_context(tc.tile_pool(name="state", bufs=1))

            wah_sb = consts.tile([128, KT, H], F32)
            wcat_sb = consts.tile([128, 9, H], F32)
            wot_sb = consts.tile([128, KT, O], F32)
            v_sb = consts.tile([128, KT], F32)
            ba_sb = consts.tile([128, KT], F32)
            bcat_sb = consts.tile([128, KT], F32)
            bo_sb = consts.tile([O, 1], F32)
            pmat_sb = consts.tile([64, Bl], F32)
            ident_sb = consts.tile([64, 64], F32)
            ones_col = consts.tile([128, 1], F32)
            ones_row = consts.tile([1, 128], F32)
            lg_all = consts.tile([O, 4, Bl], F32)
            u_sb = consts.tile([128, 2 * Bl], F32)
            zinv = consts.tile([128, Bl], F32)
            nc.sync.dma_start(out=wah_sb[:], in_=wah.ap())
            nc.sync.dma_start(out=wcat_sb[:], in_=wcat.ap())
            nc.sync.dma_start(out=wot_sb[:], in_=wo_t.ap())
            nc.sync.dma_start(out=v_sb[:], in_=v_t.ap())
            nc.sync.dma_start(out=ba_sb[:], in_=ba_t.ap())
            nc.sync.dma_start(out=bcat_sb[:], in_=bcat.ap())
            nc.sync.dma_start(out=bo_sb[:], in_=bo_t.ap())
            nc.sync.dma_start(out=pmat_sb[:], in_=pmat.ap())
            nc.sync.dma_start(out=ident_sb[:], in_=ident.ap())
            nc.vector.memset(ones_col[:], 1.0)
            nc.vector.memset(ones_row[:], 1.0)

            enc2_sb = enc2p.tile([128, Bl, 2, H], F32)
            nc.sync.dma_start(out=enc2_sb[:], in_=enc_l2.ap())

            rnn_in = state.tile([128, 9, Bl], F32)
            nc.sync.dma_start(out=rnn_in[:, 4:8, :], in_=h0_t.ap())
            nc.sync.dma_start(out=rnn_in[0:O, 8, :], in_=x0_t.ap())
            nc.vector.memset(rnn_in[O:128, 8, :], 0.0)
            hproj = state.tile([128, KT, Bl], F32)
            hx_sb = state.tile([128, KT, Bl], F32)
            zsb = state.tile([64, 1], F32)
            zrow = state.tile([1, Bl], F32)
            lgb = state.tile([Bl, O], F32)
            mx = state.tile([Bl, 1], F32)
            oh = state.tile([Bl, O], F32)

            # enc_proj = enc @ Wa_e + ba  ->  epdram
            with tc.tile_pool(name="pre", bufs=1) as prep, \
                 tc.tile_pool(name="pre_s", bufs=3) as pres, \
                 tc.tile_pool(name="pre_ps", bufs=4, space="PSUM") as preps:
                wae_sb = prep.tile([128, KT, H], F32)
                nc.sync.dma_start(out=wae_sb[:], in_=wae.ap())
                for j in range(Bl // 2):
                    e1 = pres.tile([128, KT, 2, S], F32, tag="e1")
                    nc.sync.dma_start(out=e1[:], in_=enc_l1.ap()[:, :, 2 * j:2 * j + 2, :])
                    for mo in range(4):
                        pps = preps.tile([128, 512], F32, tag="pps")
                        for kt in range(KT):
                            nc.tensor.matmul(
                                pps[:],
                                wae_sb[:, kt, mo * 128:(mo + 1) * 128],
                                e1[:, kt, :, :].rearrange("p b s -> p (b s)"),
                                start=(kt == 0), stop=(kt == KT - 1))
                        eo = pres.tile([128, 512], F32, tag="eo")
                        nc.vector.tensor_scalar_add(eo[:], pps[:], ba_sb[:, mo:mo + 1])
                        nc.sync.dma_start(
                            out=epdram.ap()[:, 2 * j:2 * j + 2, mo, :],
                            in_=eo[:].rearrange("p (b s) -> p b s", b=2))

            ps_sc = ctx.enter_context(tc.tile_pool(name="ps_sc", bufs=1, space="PSUM"))
            ps_z = ctx.enter_context(tc.tile_pool(name="ps_z", bufs=1, space="PSUM"))
            ps_ctx = ctx.enter_context(tc.tile_pool(name="ps_c", bufs=1, space="PSUM"))
            ps_mm = ctx.enter_context(tc.tile_pool(name="ps_m", bufs=2, space="PSUM"))
            ps_sm = ctx.enter_context(tc.tile_pool(name="ps_sm", bufs=2, space="PSUM"))
            stream = ctx.enter_context(tc.tile_pool(name="stream", bufs=6))

            for t in range(steps):
                # --- hproj = Wa_h^T h ; hx = W_hh h + W_ih[:, :O] x ---
                hp_ps = ps_mm.tile([128, KT, Bl], F32, tag="mm")
                for mo in range(4):
                    for kt in range(KT):
                        nc.tensor.matmul(
                            hp_ps[:, mo, :],
                            wah_sb[:, kt, mo * 128:(mo + 1) * 128],
                            rnn_in[:, 4 + kt, :],
                            start=(kt == 0), stop=(kt == KT - 1))
                nc.vector.tensor_copy(hproj[:], hp_ps[:])

                hx_ps = ps_mm.tile([128, KT, Bl], F32, tag="mm")
                for mo in range(4):
                    for kt in (4, 5, 6, 7, 8):
                        nc.tensor.matmul(
                            hx_ps[:, mo, :],
                            wcat_sb[:, kt, mo * 128:(mo + 1) * 128],
                            rnn_in[:, kt, :],
                            start=(kt == 4), stop=(kt == 8))
                nc.vector.tensor_copy(hx_sb[:], hx_ps[:])

                # --- chunk pipeline: energy -> tanh -> score -----------------
                # processing position pos covers column-batches c = 2*pos+bi;
                # batch b = (4 + c) % 32 so the two res_ep chunks (b 0..3) are
                # processed LAST -- stream-ring buffers then free early enough
                # for 3 prefetch DMAs of step t+1 to overlap this step's tail.
                sc_ps = [ps_sc.tile([128, 512], F32, tag="scA", name="scA"),
                         ps_sc.tile([128, 512], F32, tag="scB", name="scB")]
                ctx_ps = ps_ctx.tile([128, KT, Bl], F32, tag="ctx")
                z_ps = ps_z.tile([64, 1], F32, tag="z")
                for pos in range(16):
                    g = pos // 8
                    w = stream.tile([128, 2, KT, S], F32, tag="ep")
                    nc.sync.dma_start(out=w[:],
                                      in_=epdram.ap()[:, 2 * pos:2 * pos + 2, :, :])
                    src_ap = w[:]
                    for bi in range(2):
                        c = 2 * pos + bi
                        b = c
                        for kt in range(KT):
                            nc.vector.tensor_scalar_add(
                                w[:, bi, kt, :], src_ap[:, bi, kt, :],
                                hproj[:, kt, b:b + 1])
                    nc.scalar.activation(w[:], w[:], AF.Tanh)
                    for bi in range(2):
                        c = 2 * pos + bi
                        col = 4 * (pos % 8) + 2 * bi
                        for s1 in range(2):
                            for kt in range(KT):
                                nc.tensor.matmul(
                                    sc_ps[g][:, col + s1:col + s1 + 1],
                                    w[:, bi, kt, s1 * 128:(s1 + 1) * 128],
                                    v_sb[:, kt:kt + 1],
                                    start=(kt == 0), stop=(kt == KT - 1))
                    if pos == 7 or pos == 15:
                        # group complete: exp, Z, zinv, context, evac
                        nc.scalar.activation(u_sb[:, 32 * g:32 * g + 32],
                                             sc_ps[g][:, 0:32], AF.Exp)
                        nc.tensor.matmul(z_ps[32 * g:32 * g + 32, :],
                                         u_sb[:, 32 * g:32 * g + 32], ones_col[:],
                                         start=True, stop=True,
                                         tile_position=(0, 32 * g))
                        nc.vector.tensor_copy(zsb[32 * g:32 * g + 32, :],
                                              z_ps[32 * g:32 * g + 32, :])
                        zrow_ps = ps_sm.tile([1, 16], F32, tag="sm")
                        nc.tensor.matmul(zrow_ps[:],
                                         zsb[32 * g:32 * g + 32, :],
                                         pmat_sb[32 * g:32 * g + 32,
                                                 16 * g:16 * g + 16],
                                         start=True, stop=True)
                        nc.vector.tensor_copy(zrow[:, 16 * g:16 * g + 16],
                                              zrow_ps[:])
                        zrep_ps = ps_sm.tile([128, 16], F32, tag="sm")
                        nc.tensor.matmul(zrep_ps[:], ones_row[:],
                                         zrow[:, 16 * g:16 * g + 16],
                                         start=True, stop=True)
                        nc.vector.reciprocal(zinv[:, 16 * g:16 * g + 16],
                                             zrep_ps[:])
                        for cc in range(16 * g, 16 * g + 16):
                            b = cc
                            for hc in range(4):
                                for s1 in range(2):
                                    nc.tensor.matmul(
                                        ctx_ps[:, hc, cc:cc + 1],
                                        enc2_sb[:, b, s1, hc * 128:(hc + 1) * 128],
                                        u_sb[:, 2 * cc + s1:2 * cc + s1 + 1],
                                        start=(s1 == 0), stop=(s1 == 1))
                        # evac + normalize: c runs -> b runs ((4+c)%32)
                        for kt in range(KT):
                            nc.vector.tensor_tensor(
                                out=rnn_in[:, kt, 16 * g:16 * g + 16],
                                in0=ctx_ps[:, kt, 16 * g:16 * g + 16],
                                in1=zinv[:, 16 * g:16 * g + 16], op=ALU.mult)

                # --- RNN ---
                h_ps = ps_mm.tile([128, KT, Bl], F32, tag="mm")
                for mo in range(4):
                    for kt in (0, 1, 2, 3):
                        nc.tensor.matmul(
                            h_ps[:, mo, :],
                            wcat_sb[:, kt, mo * 128:(mo + 1) * 128],
                            rnn_in[:, kt, :],
                            start=(kt == 0), stop=(kt == 3))
                nc.vector.tensor_tensor(out=h_ps[:], in0=h_ps[:], in1=hx_sb[:],
                                        op=ALU.add)
                for mo in range(4):
                    nc.scalar.activation(rnn_in[:, 4 + mo, :], h_ps[:, mo, :],
                                         AF.Tanh, bias=bcat_sb[:, mo:mo + 1])

                # --- logits ---
                te = t % T  # bench builds may run steps > T; wrap the output
                lg_ps = ps_sm.tile([O, Bl], F32, tag="sm")
                for kt in range(KT):
                    nc.tensor.matmul(lg_ps[:], wot_sb[:, kt, :], rnn_in[:, 4 + kt, :],
                                     start=(kt == 0), stop=(kt == KT - 1))
                nc.vector.tensor_scalar_add(lg_all[:, te % 4, :], lg_ps[:], bo_sb[:])
                if te % 4 == 3 or te == T - 1 or t == steps - 1:
                    t0 = (te // 4) * 4
                    nc.sync.dma_start(out=out.ap()[:, t0:te + 1, :],
                                      in_=lg_all[:, 0:te + 1 - t0, :])

                # --- greedy one-hot(argmax) feedback ---
                if t < steps - 1:
                    lgb_ps = ps_sm.tile([Bl, O], F32, tag="sm")
                    nc.tensor.transpose(lgb_ps[:], lg_all[:, te % 4, :],
                                        ident_sb[0:O, 0:O])
                    nc.vector.tensor_copy(lgb[:], lgb_ps[:])
                    nc.vector.tensor_reduce(mx[:], lgb[:], axis=mybir.AxisListType.X,
                                            op=ALU.max)
                    nc.vector.tensor_scalar(out=oh[:], in0=lgb[:], scalar1=mx[:],
                                            scalar2=None, op0=ALU.is_equal)
                    oh_ps = ps_sm.tile([O, Bl], F32, tag="sm")
                    nc.tensor.transpose(oh_ps[:], oh[:], ident_sb[0:Bl, 0:Bl])
                    nc.vector.tensor_copy(rnn_in[0:O, 8, :], oh_ps[:])

    nc.compile()
    return nc


def _prep_core_inputs(enc, h0, x0, Wa, ba, v, W_ih, b_ih, W_hh, b_hh, Wo, bo):
    f = np.float32
    Wa_h, Wa_e = Wa[:H], Wa[H:]
    enc_l1 = np.ascontiguousarray(
        enc.transpose(2, 1, 0).reshape(KT, 128, Bl, S).transpose(1, 0, 2, 3), dtype=f)
    enc_l2 = np.ascontiguousarray(
        enc.reshape(2, 128, Bl, H).transpose(1, 2, 0, 3), dtype=f)
    h0_t = np.ascontiguousarray(h0.T.reshape(KT, 128, Bl).transpose(1, 0, 2), dtype=f)
    x0_t = np.ascontiguousarray(x0.T, dtype=f)
    wae_ = np.ascontiguousarray(Wa_e.reshape(KT, 128, H).transpose(1, 0, 2), dtype=f)
    wah_ = np.ascontiguousarray(Wa_h.reshape(KT, 128, H).transpose(1, 0, 2), dtype=f)
    Wcat = np.zeros((9 * 128, H), dtype=f)
    Wcat[0:H] = W_ih[:, O:O + H].T
    Wcat[H:2 * H] = W_hh.T
    Wcat[2 * H:2 * H + O] = W_ih[:, 0:O].T
    wcat_ = np.ascontiguousarray(Wcat.reshape(9, 128, H).transpose(1, 0, 2), dtype=f)
    wo_ = np.ascontiguousarray(Wo.T.reshape(KT, 128, O).transpose(1, 0, 2), dtype=f)
    v_ = np.ascontiguousarray(np.asarray(v, dtype=f).reshape(KT, 128).T)
    ba_ = np.ascontiguousarray(np.asarray(ba, dtype=f).reshape(KT, 128).T)
    bc = (np.asarray(b_ih, dtype=f) + np.asarray(b_hh, dtype=f))
    bcat_ = np.ascontiguousarray(bc.reshape(KT, 128).T)
    bo_ = np.ascontiguousarray(np.asarray(bo, dtype=f).reshape(O, 1))
    # z partials: group g (batches 16g..16g+15) lands on partitions
    # 32g + 2*j + s1 (j = local batch); pmat combines the s1 pairs.
    pm = np.zeros((64, Bl), dtype=f)
    for g in range(2):
        for j in range(16):
            for s1 in range(2):
                pm[32 * g + 2 * j + s1, 16 * g + j] = 1.0
    return {
        "enc_l1": enc_l1, "enc_l2": enc_l2, "h0_t": h0_t, "x0_t": x0_t,
        "wae": wae_, "wah": wah_, "wcat": wcat_, "wo_t": wo_, "v_t": v_,
        "ba_t": ba_, "bcat": bcat_, "bo_t": bo_, "pmat": pm,
        "ident": np.eye(64, dtype=f),
    }


_NC_CACHE = {}


def _get_nc():
    if "nc" not in _NC_CACHE:
        _NC_CACHE["nc"] = _build()
    return _NC_CACHE["nc"]


def kernel(sos_token, h, encoder_outputs, Wa, ba, v, W_ih, b_ih, W_hh, b_hh,
           Wo, bo):
    sos_token = np.asarray(sos_token, dtype=np.float32)
    h = np.asarray(h, dtype=np.float32)
    encoder_outputs = np.asarray(encoder_outputs, dtype=np.float32)
    nc = _get_nc()
    in_maps = []
    for core in range(NCORES):
        sl = slice(core * Bl, (core + 1) * Bl)
        in_maps.append(_prep_core_inputs(
            encoder_outputs[:, sl], h[0][sl], sos_token[0][sl],
            Wa, ba, v, W_ih, b_ih, W_hh, b_hh, Wo, bo))
    res = bass_utils.run_bass_kernel_spmd(nc, in_maps, core_ids=list(range(NCORES)))
    # per-core out [O, T, Bl] -> full [B, O, T]
    return np.concatenate(
        [res.results[c]["out"].transpose(2, 0, 1) for c in range(NCORES)], axis=0)
